# revision 21
# baseline (speedup 1.0000x reference)
"""Single-layer transformer LM head kernel for 8 Trainium2 NeuronCores.

Model (B=2, T=2048, D=1024, V=32000):
    x = tok_emb[idx] + pos_emb
    x = x + 0.125 * causal_attn(x@Wq, x@Wk, x@Wv)
    x = x + gelu(x@W1 + b1)@W2 + b2
    out = x@Wout + bout

Sharding (one uniform SPMD program on 8 cores):
  - trunk token-parallel: core c owns 512 tokens (batch c//4, block c%4)
  - K/V for the whole batch-sequence are recomputed locally on every core
    (cheaper than the 4-rank AllGather, measured): the host feeds each core
    the full-batch embeddings with the 512-token blocks ROTATED so the
    core's own block is always first -- this keeps every access pattern
    static/uniform across cores; causality lives in a per-core additive
    mask input built for the rotated order.
  - final-hidden AllGather across all 8 cores, split in two halves so the
    first half overlaps the tail of the MLP down-projection
  - logits vocab-parallel: each core does all 4096 tokens x 4000 vocab cols

All activations are kept in transposed [d, tokens] layout so every matmul
(lhsT.T @ rhs, contraction on partitions) is expressed without on-chip
transposes; attention scores are computed transposed [tk, tq] so the
softmax normalizer comes from a ones-vector matmul.

Matmul operands, the final-hidden AllGather, and all weight inputs are
bfloat16 (same PE rate as fp32r, half the HBM/DMA and SBUF traffic);
accumulation (PSUM), softmax statistics, residual masters and the logits
output stay fp32.  K and V stay resident in SBUF in exactly the layout
attention consumes (K-proj emits [d, tok], V-proj emits [tok, d]) -- no
DRAM round-trip.  Host folds pos_emb into the token embeddings, so the
kernel's embedding phase is a pure DMA.
"""
import numpy as np
import ml_dtypes
import concourse.bass as bass
import concourse.bacc as bacc
import concourse.tile as tile
from concourse import bass_utils, mybir
from contextlib import ExitStack

F32 = mybir.dt.float32
BF = mybir.dt.bfloat16
AF = mybir.ActivationFunctionType
OP = mybir.AluOpType
NPBF = ml_dtypes.bfloat16

N_CORES = 8
B, T, D, DH, V = 2, 2048, 1024, 4096, 32000
TB = T // 4            # 512 tokens per core
VS = V // N_CORES      # 4000 vocab cols per core
VT = VS // 8           # 500 per n-tile
NV = V // VT           # 64 vocab tiles (full vocab, token-parallel logits)
KC = D // 128          # 8 contraction chunks of d_model
HC = DH // 128         # 32 chunks of d_hidden
NTK = T // 128         # 16 key chunks (whole batch-sequence)
SCALE = 1.0 / 32.0     # 1/sqrt(D)
MASK_NEG = -1.0e4
XH_ELEMS = (KC // 2) * 128 * TB   # half of one core's final-hidden block

KV_GROUPS = [[0, 1, 2, 3], [4, 5, 6, 7]]

_STATE = {}
_NO_COLL = False   # timing/sim variant: skip collectives


def _alloc_logits_pools(tc, st, lg):
    """Reserve the logits staging pools on the right SBUF side (so trunk
    pools on the left can pop in stack order underneath them).  x2bf (the
    final hidden, bf16) also lives here so it survives into _logits."""
    lg["wop"] = st.enter_context(tc.tile_pool(name="wop", bufs=3,
                                              side="right"))
    lg["bop"] = st.enter_context(tc.tile_pool(name="bop", bufs=4,
                                              side="right"))
    x2p = st.enter_context(tc.tile_pool(name="x2p", bufs=1, side="right"))
    lg["wot0"] = lg["wop"].tile([128, KC, VT], BF, name="wot")
    lg["bout0"] = lg["bop"].tile([128, VT], F32, name="bout_bc")
    lg["x2bf"] = x2p.tile([128, KC, TB], BF, name="x2bf")


def _load_logits0(nc, io, lg):
    """Queue the n=0 logits weight loads; called at MLP start so the loads
    land well before the trunk finishes."""
    nc.scalar.dma_start(
        lg["bout0"][:], io["boutb"].ap()[0:1, :].partition_broadcast(128))
    nc.sync.dma_start(lg["wot0"][:], io["woutb"].ap()[0])


def _trunk(nc, tc, io, dp, bounce_k, bounce_v, ag_k, ag_v, st, lg):
    """Token-parallel trunk; ends with the split final-hidden AllGather.

    Pool lifetimes are managed manually: kv/res0 (K,V,Q + x0 master) are
    released right after attention so the MLP fits; res1 (x1 masters)
    spans attention through the MLP."""
    kvp = tc.alloc_tile_pool(name="kv", bufs=1)
    r0p = tc.alloc_tile_pool(name="res0", bufs=1)
    if True:
        kT_s = kvp.tile([128, KC, T], BF)      # K in [d, tok] layout
        v_s = kvp.tile([128, NTK, D], BF)      # V in [tok, d] layout
        qT = kvp.tile([128, KC, TB], BF)
        x0f32 = r0p.tile([128, KC, TB], F32)
        rs_b = r0p.tile([128, TB], F32)

        # ---- own-block embeddings + K/V/Q projections.  K/V for the other
        # three blocks arrive via two 4-rank AllGathers that fly behind the
        # V/Q projections and the early scores iterations ----
        with tc.tile_pool(name="xfull", bufs=1) as xp, \
             tc.tile_pool(name="kvown", bufs=1) as kop:
            x0F = xp.tile([128, KC, TB], BF)
            kt_own = kop.tile([128, KC, TB], BF)
            v_own = kop.tile([128, 4, D], BF)
            with tc.tile_pool(name="wqp", bufs=1) as wqp:
                wq_s = wqp.tile([128, KC, KC, 128], BF)
                with tc.tile_pool(name="wvp", bufs=1) as wvp, \
                     tc.tile_pool(name="wkp", bufs=1) as wkp, \
                     tc.tile_pool(name="ps_kv", bufs=6, space="PSUM") as pskv:
                    wk_s = wkp.tile([128, KC, KC, 128], BF)
                    wvr_s = wvp.tile([128, KC, D], BF)
                    # first matmul needs xt + wk: queue those first,
                    # wvr/wq on the other HWDGE queue
                    nc.sync.dma_start(
                        x0F[:], io["xt"].ap().rearrange("k p t -> p k t"))
                    for m in range(KC):
                        nc.scalar.dma_start(wk_s[:, m], io["wk"].ap()[:, m])
                    nc.sync.dma_start(wvr_s[:], io["wvr"].ap())
                    nc.scalar.dma_start(wq_s[:], io["wq"].ap())

                    # K projection (own block) -> bounce -> AllGather
                    for m in range(KC):
                        ps = pskv.tile([128, TB], F32, name="ps_kv")
                        for k in range(KC):
                            nc.tensor.matmul(
                                ps[:], wk_s[:, m, k, :], x0F[:, k, :],
                                start=(k == 0), stop=(k == KC - 1))
                        nc.vector.tensor_copy(kt_own[:, m, :], ps[:])
                    nc.scalar.dma_start(
                        bounce_k[:].rearrange("(k p t) -> p k t",
                                              k=KC, p=128),
                        kt_own[:])
                    if not _NO_COLL:
                        nc.gpsimd.collective_compute(
                            "AllGather", OP.bypass, replica_groups=KV_GROUPS,
                            ins=[bounce_k.opt()], outs=[ag_k.opt()])

                    # V projection (own block) -> bounce -> AllGather
                    for tc_ in range(4):
                        for h in range(2):
                            ps = pskv.tile([128, 512], F32, name="ps_kv")
                            for k in range(KC):
                                nc.tensor.matmul(
                                    ps[:], x0F[:, k, bass.ts(tc_, 128)],
                                    wvr_s[:, k, bass.ts(h, 512)],
                                    start=(k == 0), stop=(k == KC - 1))
                            nc.vector.tensor_copy(
                                v_own[:, tc_, bass.ts(h, 512)], ps[:])
                    nc.scalar.dma_start(
                        bounce_v[:].rearrange("(c p d) -> p c d", c=4, p=128),
                        v_own[:])
                    if not _NO_COLL:
                        nc.gpsimd.collective_compute(
                            "AllGather", OP.bypass, replica_groups=KV_GROUPS,
                            ins=[bounce_v.opt()], outs=[ag_v.opt()])

                    # Q projection (own block) while the AllGathers fly
                    for m in range(KC):
                        ps = pskv.tile([128, TB], F32, name="ps_kv")
                        for k in range(KC):
                            nc.tensor.matmul(
                                ps[:], wq_s[:, m, k, :], x0F[:, k, :],
                                start=(k == 0), stop=(k == KC - 1))
                        nc.vector.tensor_copy(qT[:, m, :], ps[:])
                    nc.vector.tensor_copy(x0f32[:], x0F[:])
                    # pre-load the ACT engine's Exp table so the scores
                    # loop doesn't stall on LoadActFuncSet
                    warm = r0p.tile([128, 1], F32, name="warm")
                    nc.vector.memset(warm[:], 0.0)
                    nc.scalar.activation(warm[:], warm[:], AF.Exp,
                                         scale=1.0)

            # gathered K/V -> SBUF (group member i = global block i)
            for i in range(4):
                nc.sync.dma_start(
                    kT_s[:, :, bass.ts(i, TB)],
                    ag_k[i].rearrange("(k p t) -> p k t", k=KC, p=128))
            for i in range(4):
                nc.sync.dma_start(
                    v_s[:, 4 * i:4 * i + 4, :],
                    ag_v[i].rearrange("(c p d) -> p c d", c=4, p=128))

        # ---------- attention (scores transposed: sT[tk, tq]) ----------
        if lg is not None:
            _alloc_logits_pools(tc, st, lg)
        r1p = tc.alloc_tile_pool(name="res1", bufs=1, side="right")
        x1bf = r1p.tile([128, KC, TB], BF)
        x1f32 = r1p.tile([128, KC, TB], F32)
        with tc.tile_pool(name="attn", bufs=1) as ap_, \
             tc.tile_pool(name="mskp", bufs=6) as mskp, \
             tc.tile_pool(name="stmp", bufs=3) as stp, \
             tc.tile_pool(name="ps_sc", bufs=2, space="PSUM") as ps_sc, \
             tc.tile_pool(name="ps_l", bufs=1, space="PSUM") as ps_lp, \
             tc.tile_pool(name="ps_o", bufs=2, space="PSUM") as ps_op:
            attnT = ap_.tile([128, NTK, TB], BF)
            ones_f32 = ap_.tile([128, 1], F32)
            nc.vector.memset(ones_f32[:], 1.0)
            ones_s = ap_.tile([128, 1], BF)
            nc.vector.tensor_copy(ones_s[:], ones_f32[:])
            ps_l = ps_lp.tile([1, TB], F32)

            for tkc in range(NTK):
                msk = mskp.tile([128, TB], F32, name="msk")
                nc.sync.dma_start(msk[:], io["mask"].ap()[tkc])
                ps = ps_sc.tile([128, TB], F32, name="ps_s")
                for k in range(KC):
                    nc.tensor.matmul(ps[:], kT_s[:, k, bass.ts(tkc, 128)],
                                     qT[:, k, :],
                                     start=(k == 0), stop=(k == KC - 1))
                stmp = stp.tile([128, TB], F32, name="stmp")
                nc.vector.tensor_tensor(out=stmp[:], in0=ps[:], in1=msk[:],
                                        op=OP.add)
                nc.scalar.activation(attnT[:, tkc, :], stmp[:], AF.Exp,
                                     scale=SCALE)
                nc.tensor.matmul(ps_l[:], ones_s[:], attnT[:, tkc, :],
                                 start=(tkc == 0), stop=(tkc == NTK - 1))

            # rs = 0.125 / l, broadcast to all partitions via DRAM bounce
            rs_row = ap_.tile([1, TB], F32)
            nc.vector.reciprocal(rs_row[:], ps_l[:])
            rs_row2 = ap_.tile([1, TB], F32)
            nc.vector.tensor_scalar_mul(rs_row2[:], rs_row[:], 0.125)
            rs_dram = dp.tile([1, TB], F32, name="rs_dram")
            nc.sync.dma_start(rs_dram[:], rs_row2[:])
            nc.sync.dma_start(rs_b[:], rs_dram[:].partition_broadcast(128))

            # oT[dv, tq] = V.T @ attnT ; x1 = x0 + rs * oT
            for m in range(KC):
                ps = ps_op.tile([128, TB], F32, name="ps_av")
                for tkc in range(NTK):
                    nc.tensor.matmul(ps[:], v_s[:, tkc, bass.ts(m, 128)],
                                     attnT[:, tkc, :],
                                     start=(tkc == 0), stop=(tkc == NTK - 1))
                ot = stp.tile([128, TB], F32, name="otmp")
                nc.vector.tensor_tensor(out=ot[:], in0=ps[:], in1=rs_b[:],
                                        op=OP.mult)
                nc.vector.tensor_tensor(out=x1f32[:, m, :], in0=ot[:],
                                        in1=x0f32[:, m, :], op=OP.add)
                nc.vector.tensor_copy(x1bf[:, m, :], x1f32[:, m, :])

        r0p.release()
        kvp.release()

        # ---------- MLP ----------
        with tc.tile_pool(name="mlp", bufs=1, side="right") as mp, \
             tc.tile_pool(name="w1p", bufs=4, side="right") as w1p, \
             tc.tile_pool(name="w2p", bufs=2, side="right") as w2p, \
             tc.tile_pool(name="ps_h", bufs=6, space="PSUM") as ps_hp:
            if lg is not None:
                _load_logits0(nc, io, lg)
            b1_s = mp.tile([128, HC], F32)
            b2_s = mp.tile([128, KC], F32)
            nc.sync.dma_start(b1_s[:], io["b1t"].ap())
            nc.sync.dma_start(b2_s[:], io["b2t"].ap())
            hT = mp.tile([128, HC, TB], BF)
            for m in range(HC):
                w1t = w1p.tile([128, KC, 128], BF, name="w1t")
                nc.sync.dma_start(w1t[:], io["w1b"].ap()[m])
                ps = ps_hp.tile([128, TB], F32, name="ps_mlp")
                for k in range(KC):
                    nc.tensor.matmul(ps[:], w1t[:, k, :], x1bf[:, k, :],
                                     start=(k == 0), stop=(k == KC - 1))
                nc.scalar.activation(hT[:, m, :], ps[:], AF.Gelu,
                                     bias=b1_s[:, m:m + 1], scale=1.0)
            x2bf = lg["x2bf"] if lg is not None else mp.tile([128, KC, TB], BF)
            for m in range(KC):
                w2t = w2p.tile([128, HC, 128], BF, name="w2t")
                nc.scalar.dma_start(w2t[:], io["w2b"].ap()[m])
                ps = ps_hp.tile([128, TB], F32, name="ps_mlp")
                for k in range(HC):
                    nc.tensor.matmul(ps[:], w2t[:, k, :], hT[:, k, :],
                                     start=(k == 0), stop=(k == HC - 1))
                # x2 = (psum + b2) + x1
                nc.vector.scalar_tensor_tensor(
                    out=x2bf[:, m, :], in0=ps[:], scalar=b2_s[:, m:m + 1],
                    in1=x1f32[:, m, :], op0=OP.add, op1=OP.add)
        r1p.release()


def _logits(nc, tc, io, dp, st, lg):
    """Token-parallel logits: each core does its own 512 tokens x the FULL
    vocab from its SBUF-resident final hidden -- no collective at all.
    Output is written bf16 (host upcasts); same FLOPs as vocab-parallel."""
    out_d = io["logits"]
    if lg is None:
        lg = {}
        _alloc_logits_pools(tc, st, lg)
        _load_logits0(nc, io, lg)
    wop, bop, x2bf = lg["wop"], lg["bop"], lg["x2bf"]
    with tc.tile_pool(name="outp", bufs=4) as outp, \
         tc.tile_pool(name="ps_lg", bufs=8, space="PSUM") as ps_lg:
        wot, bout_bc = lg["wot0"], lg["bout0"]
        for n in range(NV):
            if n > 0:
                wot = wop.tile([128, KC, VT], BF, name="wot")
                nc.sync.dma_start(wot[:], io["woutb"].ap()[n])
                bout_bc = bop.tile([128, VT], F32, name="bout_bc")
                nc.scalar.dma_start(
                    bout_bc[:],
                    io["boutb"].ap()[n:n + 1, :].partition_broadcast(128))
            for t4 in range(4):
                ps = ps_lg.tile([128, VT], F32, name="ps_g")
                for k in range(KC):
                    nc.tensor.matmul(
                        ps[:], x2bf[:, k, bass.ts(t4, 128)], wot[:, k, :],
                        start=(k == 0), stop=(k == KC - 1))
                ot = outp.tile([128, VT], BF, name="og")
                nc.vector.tensor_tensor(out=ot[:], in0=ps[:],
                                        in1=bout_bc[:], op=OP.add)
                nc.scalar.dma_start(
                    out_d.ap()[n, bass.ts(t4, 128), :], ot[:])


def _build(repeat=1, phases="full"):
    nc = bacc.Bacc("TRN2", target_bir_lowering=False, debug=False,
                   num_devices=N_CORES)

    # ---- kernel I/O (per-core shards prepared on host) ----
    io = {}
    def inp(name, shape, dt=BF):
        io[name] = nc.dram_tensor(name, shape, dt, kind="ExternalInput")
    inp("xt", [KC, 128, TB])
    inp("wq", [128, KC, KC, 128])
    inp("wk", [128, KC, KC, 128])
    inp("wvr", [128, KC, D])
    inp("w1b", [HC, 128, KC, 128])
    inp("b1t", [128, HC], F32)
    inp("w2b", [KC, 128, HC, 128])
    inp("b2t", [128, KC], F32)
    inp("woutb", [NV, 128, KC, VT])
    inp("boutb", [NV, VT], F32)
    inp("mask", [NTK, 128, TB], F32)
    io["logits"] = nc.dram_tensor("logits", [NV, TB, VT], BF,
                                  kind="ExternalOutput")

    with tile.TileContext(nc) as tc:
        with tc.tile_pool(name="dram", bufs=1, space="DRAM") as dp:
            for _ in range(repeat):  # repeat>1 is a timing-only variant
                bounce_k = dp.tile([KC * 128 * TB], BF, name="bounce_k")
                bounce_v = dp.tile([4 * 128 * D], BF, name="bounce_v")
                ag_k = dp.tile([4, KC * 128 * TB], BF, name="ag_k",
                               addr_space="Shared")
                ag_v = dp.tile([4, 4 * 128 * D], BF, name="ag_v",
                               addr_space="Shared")
                with ExitStack() as st:
                    lg = {} if phases == "full" else None
                    if phases in ("full", "trunk"):
                        _trunk(nc, tc, io, dp, bounce_k, bounce_v,
                               ag_k, ag_v, st, lg)
                    if phases in ("full", "logits"):
                        _logits(nc, tc, io, dp, st, lg)

    nc.compile()
    return nc


def _prep_shared(Wq, Wk, Wv, W1, b1, W2, b2, pos_emb, Wout, bout):
    f = np.float32
    sh = {}
    sh["wq"] = np.ascontiguousarray(
        Wq.reshape(KC, 128, KC, 128).transpose(1, 2, 0, 3), dtype=NPBF)
    sh["wk"] = np.ascontiguousarray(
        Wk.reshape(KC, 128, KC, 128).transpose(1, 2, 0, 3), dtype=NPBF)
    sh["wvr"] = np.ascontiguousarray(
        Wv.reshape(KC, 128, D).transpose(1, 0, 2), dtype=NPBF)
    sh["w1b"] = np.ascontiguousarray(
        W1.reshape(KC, 128, HC, 128).transpose(2, 1, 0, 3), dtype=NPBF)
    sh["b1t"] = np.ascontiguousarray(b1.reshape(HC, 128).T, dtype=f)
    sh["w2b"] = np.ascontiguousarray(
        W2.reshape(HC, 128, KC, 128).transpose(2, 1, 0, 3), dtype=NPBF)
    sh["b2t"] = np.ascontiguousarray(b2.reshape(KC, 128).T, dtype=f)
    sh["woutb"] = np.ascontiguousarray(
        Wout.reshape(KC, 128, NV, VT).transpose(2, 1, 0, 3), dtype=NPBF)
    sh["boutb"] = np.ascontiguousarray(bout.reshape(NV, VT), dtype=f)

    # global-order causal masks: core with block j has queries
    # TB*j + cc; key chunk tkc covers global keys 128*tkc + rr.
    pos = np.asarray(pos_emb[:T], dtype=f)
    masks = []
    rr = np.arange(128)[:, None]
    cc = np.arange(TB)[None, :]
    for j in range(4):
        m = np.empty((NTK, 128, TB), dtype=f)
        for tkc in range(NTK):
            gtk = 128 * tkc + rr
            m[tkc] = np.where(gtk <= TB * j + cc, 0.0, MASK_NEG)
        masks.append(m)
    return sh, pos, masks


def make_in_maps(idx, tok_emb, pos_emb, Wq, Wk, Wv, W1, b1, W2, b2,
                 Wout, bout):
    f = np.float32
    tok_emb = np.asarray(tok_emb, dtype=f)
    idx = np.asarray(idx)
    sh, pos, masks = _prep_shared(
        np.asarray(Wq, f), np.asarray(Wk, f), np.asarray(Wv, f),
        np.asarray(W1, f), np.asarray(b1, f), np.asarray(W2, f),
        np.asarray(b2, f), np.asarray(pos_emb, f), np.asarray(Wout, f),
        np.asarray(bout, f))

    tok_full = [tok_emb[np.asarray(idx[b], dtype=np.int64)] for b in range(B)]
    in_maps = []
    for c in range(N_CORES):
        b, j = c // 4, c % 4
        own = tok_full[b][TB * j:TB * (j + 1)] + pos[TB * j:TB * (j + 1)]
        m = dict(sh)
        m["xt"] = np.ascontiguousarray(own.T.reshape(KC, 128, TB), dtype=NPBF)
        m["mask"] = masks[j]
        in_maps.append(m)
    return in_maps


def kernel(idx, tok_emb, pos_emb, Wq, Wk, Wv, W1, b1, W2, b2, Wout, bout):
    if "nc" not in _STATE:
        _STATE["nc"] = _build()
    nc = _STATE["nc"]

    in_maps = make_in_maps(idx, tok_emb, pos_emb, Wq, Wk, Wv, W1, b1, W2,
                           b2, Wout, bout)
    res = bass_utils.run_bass_kernel_spmd(nc, in_maps,
                                          core_ids=list(range(N_CORES)))
    _STATE["last_results"] = res

    out = np.empty((B * T, V), dtype=np.float32)
    for c in range(N_CORES):
        lg = res.results[c]["logits"]             # [64, 512, 500] bf16
        out[TB * c:TB * (c + 1), :] = (
            np.asarray(lg).transpose(1, 0, 2).reshape(TB, V)
            .astype(np.float32))
    return out.reshape(B, T, V)


# revision 26
# speedup vs baseline: 1.1112x; 1.1112x over previous
"""Single-layer transformer LM head kernel for 8 Trainium2 NeuronCores.

Model (B=2, T=2048, D=1024, V=32000):
    x = tok_emb[idx] + pos_emb
    x = x + 0.125 * causal_attn(x@Wq, x@Wk, x@Wv)
    x = x + gelu(x@W1 + b1)@W2 + b2
    out = x@Wout + bout

Sharding (one uniform SPMD program on 8 cores):
  - trunk token-parallel: core c owns 512 tokens (batch c//4, block c%4)
  - K/V for the whole batch-sequence are recomputed locally on every core
    (cheaper than the 4-rank AllGather, measured): the host feeds each core
    the full-batch embeddings with the 512-token blocks ROTATED so the
    core's own block is always first -- this keeps every access pattern
    static/uniform across cores; causality lives in a per-core additive
    mask input built for the rotated order.
  - final-hidden AllGather across all 8 cores, split in two halves so the
    first half overlaps the tail of the MLP down-projection
  - logits vocab-parallel: each core does all 4096 tokens x 4000 vocab cols

All activations are kept in transposed [d, tokens] layout so every matmul
(lhsT.T @ rhs, contraction on partitions) is expressed without on-chip
transposes; attention scores are computed transposed [tk, tq] so the
softmax normalizer comes from a ones-vector matmul.

Matmul operands, the final-hidden AllGather, and all weight inputs are
bfloat16 (same PE rate as fp32r, half the HBM/DMA and SBUF traffic);
accumulation (PSUM), softmax statistics, residual masters and the logits
output stay fp32.  K and V stay resident in SBUF in exactly the layout
attention consumes (K-proj emits [d, tok], V-proj emits [tok, d]) -- no
DRAM round-trip.  Host folds pos_emb into the token embeddings, so the
kernel's embedding phase is a pure DMA.
"""
import numpy as np
import ml_dtypes
import concourse.bass as bass
import concourse.bacc as bacc
import concourse.tile as tile
from concourse import bass_utils, mybir
from contextlib import ExitStack

F32 = mybir.dt.float32
BF = mybir.dt.bfloat16
AF = mybir.ActivationFunctionType
OP = mybir.AluOpType
NPBF = ml_dtypes.bfloat16

N_CORES = 8
B, T, D, DH, V = 2, 2048, 1024, 4096, 32000
TB = T // 4            # 512 tokens per core
VS = V // N_CORES      # 4000 vocab cols per core
VT = VS // 8           # 500 per n-tile
NV = V // VT           # 64 vocab tiles (full vocab, token-parallel logits)
KC = D // 128          # 8 contraction chunks of d_model
HC = DH // 128         # 32 chunks of d_hidden
NTK = T // 128         # 16 key chunks (whole batch-sequence)
SCALE = 1.0 / 32.0     # 1/sqrt(D)
MASK_NEG = -1.0e4
XH_ELEMS = (KC // 2) * 128 * TB   # half of one core's final-hidden block

KV_GROUPS = [[0, 1, 2, 3], [4, 5, 6, 7]]

_STATE = {}
_NO_COLL = False   # timing/sim variant: skip collectives


def _alloc_logits_pools(tc, st, lg):
    """Reserve the logits staging pools on the right SBUF side (so trunk
    pools on the left can pop in stack order underneath them)."""
    lp = st.enter_context(tc.tile_pool(name="lgp", bufs=1, side="right"))
    lg["wop"] = st.enter_context(tc.tile_pool(name="wop", bufs=3,
                                              side="right"))
    lg["bout_s"] = lp.tile([128, 8, VT], F32, name="bout_s")
    lg["wot0"] = lg["wop"].tile([128, KC, VT], BF, name="wot")


def _load_logits0(nc, io, lg):
    """Queue the n=0 logits weight loads; called at MLP start so the loads
    land well before the trunk finishes."""
    for n in range(8):
        nc.scalar.dma_start(
            lg["bout_s"][:, n, :],
            io["boutb"].ap()[n:n + 1, :].partition_broadcast(128))
    nc.sync.dma_start(lg["wot0"][:], io["woutb"].ap()[0])


def _trunk(nc, tc, io, dp, bounce_k, bounce_v, ag_k, ag_v,
           bounce_x1, bounce_x2, ag_x1, ag_x2, st, lg):
    """Token-parallel trunk; ends with the split final-hidden AllGather.

    Pool lifetimes are managed manually: kv/res0 (K,V,Q + x0 master) are
    released right after attention so the MLP fits; res1 (x1 masters)
    spans attention through the MLP."""
    kvp = tc.alloc_tile_pool(name="kv", bufs=1)
    r0p = tc.alloc_tile_pool(name="res0", bufs=1)
    if True:
        kT_s = kvp.tile([128, KC, T], BF)      # K in [d, tok] layout
        v_s = kvp.tile([128, NTK, D], BF)      # V in [tok, d] layout
        qT = kvp.tile([128, KC, TB], BF)
        x0f32 = r0p.tile([128, KC, TB], F32)
        rs_b = r0p.tile([128, TB], F32)

        # ---- own-block embeddings + K/V/Q projections.  K/V for the other
        # three blocks arrive via two 4-rank AllGathers that fly behind the
        # V/Q projections and the early scores iterations ----
        with tc.tile_pool(name="xfull", bufs=1) as xp, \
             tc.tile_pool(name="kvown", bufs=1) as kop:
            x0F = xp.tile([128, KC, TB], BF)
            kt_own = kop.tile([128, KC, TB], BF)
            v_own = kop.tile([128, 4, D], BF)
            with tc.tile_pool(name="wqp", bufs=1) as wqp:
                wq_s = wqp.tile([128, KC, KC, 128], BF)
                with tc.tile_pool(name="wvp", bufs=1) as wvp, \
                     tc.tile_pool(name="wkp", bufs=1) as wkp, \
                     tc.tile_pool(name="ps_kv", bufs=6, space="PSUM") as pskv:
                    wk_s = wkp.tile([128, KC, KC, 128], BF)
                    wvr_s = wvp.tile([128, KC, D], BF)
                    # first matmul needs xt + wk: queue those first,
                    # wvr/wq on the other HWDGE queue
                    nc.sync.dma_start(
                        x0F[:], io["xt"].ap().rearrange("k p t -> p k t"))
                    for m in range(KC):
                        nc.scalar.dma_start(wk_s[:, m], io["wk"].ap()[:, m])
                    nc.sync.dma_start(wvr_s[:], io["wvr"].ap())
                    nc.scalar.dma_start(wq_s[:], io["wq"].ap())

                    # K projection (own block) -> bounce -> AllGather
                    for m in range(KC):
                        ps = pskv.tile([128, TB], F32, name="ps_kv")
                        for k in range(KC):
                            nc.tensor.matmul(
                                ps[:], wk_s[:, m, k, :], x0F[:, k, :],
                                start=(k == 0), stop=(k == KC - 1))
                        nc.vector.tensor_copy(kt_own[:, m, :], ps[:])
                    nc.gpsimd.dma_start(
                        bounce_k[:].rearrange("(k p t) -> p k t",
                                              k=KC, p=128),
                        kt_own[:])
                    if not _NO_COLL:
                        nc.gpsimd.collective_compute(
                            "AllGather", OP.bypass, replica_groups=KV_GROUPS,
                            ins=[bounce_k.opt()], outs=[ag_k.opt()])

                    # V projection (own block) -> bounce -> AllGather
                    for tc_ in range(4):
                        for h in range(2):
                            ps = pskv.tile([128, 512], F32, name="ps_kv")
                            for k in range(KC):
                                nc.tensor.matmul(
                                    ps[:], x0F[:, k, bass.ts(tc_, 128)],
                                    wvr_s[:, k, bass.ts(h, 512)],
                                    start=(k == 0), stop=(k == KC - 1))
                            nc.vector.tensor_copy(
                                v_own[:, tc_, bass.ts(h, 512)], ps[:])
                    nc.gpsimd.dma_start(
                        bounce_v[:].rearrange("(c p d) -> p c d", c=4, p=128),
                        v_own[:])
                    if not _NO_COLL:
                        nc.gpsimd.collective_compute(
                            "AllGather", OP.bypass, replica_groups=KV_GROUPS,
                            ins=[bounce_v.opt()], outs=[ag_v.opt()])

                    # gathered K/V -> SBUF (group member i = global block
                    # i); issued now so the loads run during Q projection
                    for i in range(4):
                        nc.gpsimd.dma_start(
                            kT_s[:, :, bass.ts(i, TB)],
                            ag_k[i].rearrange("(k p t) -> p k t",
                                              k=KC, p=128))
                    for i in range(4):
                        nc.gpsimd.dma_start(
                            v_s[:, 4 * i:4 * i + 4, :],
                            ag_v[i].rearrange("(c p d) -> p c d",
                                              c=4, p=128))

                    # Q projection (own block) while the AllGathers fly
                    for m in range(KC):
                        ps = pskv.tile([128, TB], F32, name="ps_kv")
                        for k in range(KC):
                            nc.tensor.matmul(
                                ps[:], wq_s[:, m, k, :], x0F[:, k, :],
                                start=(k == 0), stop=(k == KC - 1))
                        nc.vector.tensor_copy(qT[:, m, :], ps[:])
                    nc.vector.tensor_copy(x0f32[:], x0F[:])
                    # pre-load the ACT engine's Exp table so the scores
                    # loop doesn't stall on LoadActFuncSet
                    warm = r0p.tile([128, 1], F32, name="warm")
                    nc.vector.memset(warm[:], 0.0)
                    nc.scalar.activation(warm[:], warm[:], AF.Exp,
                                         scale=1.0)

        # ---------- attention (scores transposed: sT[tk, tq]) ----------
        if lg is not None:
            _alloc_logits_pools(tc, st, lg)
        r1p = tc.alloc_tile_pool(name="res1", bufs=1, side="right")
        x1bf = r1p.tile([128, KC, TB], BF)
        x1f32 = r1p.tile([128, KC, TB], F32)
        with tc.tile_pool(name="attn", bufs=1) as ap_, \
             tc.tile_pool(name="mskp", bufs=6) as mskp, \
             tc.tile_pool(name="stmp", bufs=3) as stp, \
             tc.tile_pool(name="ps_sc", bufs=2, space="PSUM") as ps_sc, \
             tc.tile_pool(name="ps_l", bufs=1, space="PSUM") as ps_lp, \
             tc.tile_pool(name="ps_o", bufs=2, space="PSUM") as ps_op:
            attnT = ap_.tile([128, NTK, TB], BF)
            ones_f32 = ap_.tile([128, 1], F32)
            nc.vector.memset(ones_f32[:], 1.0)
            ones_s = ap_.tile([128, 1], BF)
            nc.vector.tensor_copy(ones_s[:], ones_f32[:])
            ps_l = ps_lp.tile([1, TB], F32)

            for tkc in range(NTK):
                msk = mskp.tile([128, TB], F32, name="msk")
                nc.sync.dma_start(msk[:], io["mask"].ap()[tkc])
                ps = ps_sc.tile([128, TB], F32, name="ps_s")
                for k in range(KC):
                    nc.tensor.matmul(ps[:], kT_s[:, k, bass.ts(tkc, 128)],
                                     qT[:, k, :],
                                     start=(k == 0), stop=(k == KC - 1))
                stmp = stp.tile([128, TB], F32, name="stmp")
                nc.vector.tensor_tensor(out=stmp[:], in0=ps[:], in1=msk[:],
                                        op=OP.add)
                nc.scalar.activation(attnT[:, tkc, :], stmp[:], AF.Exp,
                                     scale=SCALE)
                nc.tensor.matmul(ps_l[:], ones_s[:], attnT[:, tkc, :],
                                 start=(tkc == 0), stop=(tkc == NTK - 1))

            # rs = 0.125 / l, broadcast to all partitions via DRAM bounce
            rs_row = ap_.tile([1, TB], F32)
            nc.vector.reciprocal(rs_row[:], ps_l[:])
            rs_row2 = ap_.tile([1, TB], F32)
            nc.vector.tensor_scalar_mul(rs_row2[:], rs_row[:], 0.125)
            rs_dram = dp.tile([1, TB], F32, name="rs_dram")
            nc.sync.dma_start(rs_dram[:], rs_row2[:])
            nc.sync.dma_start(rs_b[:], rs_dram[:].partition_broadcast(128))

            # oT[dv, tq] = V.T @ attnT ; x1 = x0 + rs * oT
            for m in range(KC):
                ps = ps_op.tile([128, TB], F32, name="ps_av")
                for tkc in range(NTK):
                    nc.tensor.matmul(ps[:], v_s[:, tkc, bass.ts(m, 128)],
                                     attnT[:, tkc, :],
                                     start=(tkc == 0), stop=(tkc == NTK - 1))
                ot = stp.tile([128, TB], F32, name="otmp")
                nc.vector.tensor_tensor(out=ot[:], in0=ps[:], in1=rs_b[:],
                                        op=OP.mult)
                nc.vector.tensor_tensor(out=x1f32[:, m, :], in0=ot[:],
                                        in1=x0f32[:, m, :], op=OP.add)
                nc.vector.tensor_copy(x1bf[:, m, :], x1f32[:, m, :])

        r0p.release()
        kvp.release()

        # ---------- MLP ----------
        with tc.tile_pool(name="mlp", bufs=1, side="right") as mp, \
             tc.tile_pool(name="w1p", bufs=4, side="right") as w1p, \
             tc.tile_pool(name="w2p", bufs=2, side="right") as w2p, \
             tc.tile_pool(name="ps_h", bufs=6, space="PSUM") as ps_hp:
            if lg is not None:
                _load_logits0(nc, io, lg)
            b1_s = mp.tile([128, HC], F32)
            b2_s = mp.tile([128, KC], F32)
            nc.sync.dma_start(b1_s[:], io["b1t"].ap())
            nc.sync.dma_start(b2_s[:], io["b2t"].ap())
            hT = mp.tile([128, HC, TB], BF)
            for m in range(HC):
                w1t = w1p.tile([128, KC, 128], BF, name="w1t")
                nc.sync.dma_start(w1t[:], io["w1b"].ap()[m])
                ps = ps_hp.tile([128, TB], F32, name="ps_mlp")
                for k in range(KC):
                    nc.tensor.matmul(ps[:], w1t[:, k, :], x1bf[:, k, :],
                                     start=(k == 0), stop=(k == KC - 1))
                nc.scalar.activation(hT[:, m, :], ps[:], AF.Gelu,
                                     bias=b1_s[:, m:m + 1], scale=1.0)
            x2bf = mp.tile([128, KC, TB], BF)
            for m in range(KC):
                w2t = w2p.tile([128, HC, 128], BF, name="w2t")
                nc.scalar.dma_start(w2t[:], io["w2b"].ap()[m])
                ps = ps_hp.tile([128, TB], F32, name="ps_mlp")
                for k in range(HC):
                    nc.tensor.matmul(ps[:], w2t[:, k, :], hT[:, k, :],
                                     start=(k == 0), stop=(k == HC - 1))
                # x2 = (psum + b2) + x1
                nc.vector.scalar_tensor_tensor(
                    out=x2bf[:, m, :], in0=ps[:], scalar=b2_s[:, m:m + 1],
                    in1=x1f32[:, m, :], op0=OP.add, op1=OP.add)
                # split final-hidden AllGather: 1st half overlaps m=4..7
                if m == KC // 2 - 1:
                    nc.scalar.dma_start(
                        bounce_x1[:].rearrange("(k p t) -> p k t",
                                               k=KC // 2, p=128),
                        x2bf[:, :KC // 2, :])
                    if not _NO_COLL:
                        nc.gpsimd.collective_compute(
                            "AllGather", OP.bypass,
                            replica_groups=[list(range(N_CORES))],
                            ins=[bounce_x1.opt()], outs=[ag_x1.opt()])
            nc.scalar.dma_start(
                bounce_x2[:].rearrange("(k p t) -> p k t",
                                       k=KC // 2, p=128),
                x2bf[:, KC // 2:, :])
            if not _NO_COLL:
                nc.gpsimd.collective_compute(
                    "AllGather", OP.bypass,
                    replica_groups=[list(range(N_CORES))],
                    ins=[bounce_x2.opt()], outs=[ag_x2.opt()])
        r1p.release()


def _logits(nc, tc, io, dp, ag_x1, ag_x2, st, lg):
    """Vocab-parallel logits over the AllGathered final hidden states.
    Output is written bf16 (host upcasts)."""
    out_d = io["logits"]
    if lg is None:
        lg = {}
        _alloc_logits_pools(tc, st, lg)
        _load_logits0(nc, io, lg)
    wop, bout_s = lg["wop"], lg["bout_s"]
    with tc.tile_pool(name="xfp", bufs=1) as lp, \
         tc.tile_pool(name="outp", bufs=4) as outp, \
         tc.tile_pool(name="ps_lg", bufs=8, space="PSUM") as ps_lg:
        xf = lp.tile([128, N_CORES * KC, TB], BF)

        def load_xf(r):
            nc.sync.dma_start(
                xf[:, KC * r:KC * r + KC // 2, :],
                ag_x1[r].rearrange("(k p t) -> p k t", k=KC // 2, p=128))
            nc.sync.dma_start(
                xf[:, KC * r + KC // 2:KC * (r + 1), :],
                ag_x2[r].rearrange("(k p t) -> p k t", k=KC // 2, p=128))
        wot = lg["wot0"]
        for n in range(8):
            if n > 0:
                wot = wop.tile([128, KC, VT], BF, name="wot")
                nc.sync.dma_start(wot[:], io["woutb"].ap()[n])
            for r in range(N_CORES):
                if n == 0:
                    load_xf(r)
                for t4 in range(4):
                    ps = ps_lg.tile([128, VT], F32, name="ps_g")
                    for k in range(KC):
                        nc.tensor.matmul(
                            ps[:], xf[:, KC * r + k, bass.ts(t4, 128)],
                            wot[:, k, :],
                            start=(k == 0), stop=(k == KC - 1))
                    ot = outp.tile([128, VT], BF, name="og")
                    nc.vector.tensor_tensor(out=ot[:], in0=ps[:],
                                            in1=bout_s[:, n, :], op=OP.add)
                    nc.scalar.dma_start(
                        out_d.ap()[r, n, bass.ts(t4, 128), :], ot[:])


def _build(repeat=1, phases="full"):
    nc = bacc.Bacc("TRN2", target_bir_lowering=False, debug=False,
                   num_devices=N_CORES)

    # ---- kernel I/O (per-core shards prepared on host) ----
    io = {}
    def inp(name, shape, dt=BF):
        io[name] = nc.dram_tensor(name, shape, dt, kind="ExternalInput")
    inp("xt", [KC, 128, TB])
    inp("wq", [128, KC, KC, 128])
    inp("wk", [128, KC, KC, 128])
    inp("wvr", [128, KC, D])
    inp("w1b", [HC, 128, KC, 128])
    inp("b1t", [128, HC], F32)
    inp("w2b", [KC, 128, HC, 128])
    inp("b2t", [128, KC], F32)
    inp("woutb", [8, 128, KC, VT])
    inp("boutb", [8, VT], F32)
    inp("mask", [NTK, 128, TB], F32)
    io["logits"] = nc.dram_tensor("logits", [N_CORES, 8, TB, VT], BF,
                                  kind="ExternalOutput")

    with tile.TileContext(nc) as tc:
        with tc.tile_pool(name="dram", bufs=1, space="DRAM") as dp:
            for _ in range(repeat):  # repeat>1 is a timing-only variant
                bounce_k = dp.tile([KC * 128 * TB], BF, name="bounce_k")
                bounce_v = dp.tile([4 * 128 * D], BF, name="bounce_v")
                ag_k = dp.tile([4, KC * 128 * TB], BF, name="ag_k")
                ag_v = dp.tile([4, 4 * 128 * D], BF, name="ag_v")
                bounce_x1 = dp.tile([XH_ELEMS], BF, name="bounce_x1")
                bounce_x2 = dp.tile([XH_ELEMS], BF, name="bounce_x2")
                ag_x1 = dp.tile([N_CORES, XH_ELEMS], BF, name="ag_x1",
                                addr_space="Shared")
                ag_x2 = dp.tile([N_CORES, XH_ELEMS], BF, name="ag_x2",
                                addr_space="Shared")
                with ExitStack() as st:
                    lg = {} if phases == "full" else None
                    if phases in ("full", "trunk"):
                        _trunk(nc, tc, io, dp, bounce_k, bounce_v,
                               ag_k, ag_v, bounce_x1, bounce_x2,
                               ag_x1, ag_x2, st, lg)
                    if phases in ("full", "logits"):
                        _logits(nc, tc, io, dp, ag_x1, ag_x2, st, lg)

    nc.compile()
    return nc


def _prep_shared(Wq, Wk, Wv, W1, b1, W2, b2, pos_emb, Wout, bout):
    f = np.float32
    sh = {}
    sh["wq"] = np.ascontiguousarray(
        Wq.reshape(KC, 128, KC, 128).transpose(1, 2, 0, 3), dtype=NPBF)
    sh["wk"] = np.ascontiguousarray(
        Wk.reshape(KC, 128, KC, 128).transpose(1, 2, 0, 3), dtype=NPBF)
    sh["wvr"] = np.ascontiguousarray(
        Wv.reshape(KC, 128, D).transpose(1, 0, 2), dtype=NPBF)
    sh["w1b"] = np.ascontiguousarray(
        W1.reshape(KC, 128, HC, 128).transpose(2, 1, 0, 3), dtype=NPBF)
    sh["b1t"] = np.ascontiguousarray(b1.reshape(HC, 128).T, dtype=f)
    sh["w2b"] = np.ascontiguousarray(
        W2.reshape(HC, 128, KC, 128).transpose(2, 1, 0, 3), dtype=NPBF)
    sh["b2t"] = np.ascontiguousarray(b2.reshape(KC, 128).T, dtype=f)


    # global-order causal masks: core with block j has queries
    # TB*j + cc; key chunk tkc covers global keys 128*tkc + rr.
    pos = np.asarray(pos_emb[:T], dtype=f)
    masks = []
    rr = np.arange(128)[:, None]
    cc = np.arange(TB)[None, :]
    for j in range(4):
        m = np.empty((NTK, 128, TB), dtype=f)
        for tkc in range(NTK):
            gtk = 128 * tkc + rr
            m[tkc] = np.where(gtk <= TB * j + cc, 0.0, MASK_NEG)
        masks.append(m)
    return sh, pos, masks


def make_in_maps(idx, tok_emb, pos_emb, Wq, Wk, Wv, W1, b1, W2, b2,
                 Wout, bout):
    f = np.float32
    tok_emb = np.asarray(tok_emb, dtype=f)
    idx = np.asarray(idx)
    sh, pos, masks = _prep_shared(
        np.asarray(Wq, f), np.asarray(Wk, f), np.asarray(Wv, f),
        np.asarray(W1, f), np.asarray(b1, f), np.asarray(W2, f),
        np.asarray(b2, f), np.asarray(pos_emb, f), None, None)
    Wout = np.asarray(Wout, f)
    bout = np.asarray(bout, f)

    tok_full = [tok_emb[np.asarray(idx[b], dtype=np.int64)] for b in range(B)]
    in_maps = []
    for c in range(N_CORES):
        b, j = c // 4, c % 4
        own = tok_full[b][TB * j:TB * (j + 1)] + pos[TB * j:TB * (j + 1)]
        m = dict(sh)
        m["xt"] = np.ascontiguousarray(own.T.reshape(KC, 128, TB), dtype=NPBF)
        m["mask"] = masks[j]
        ws = Wout[:, VS * c:VS * (c + 1)]
        m["woutb"] = np.ascontiguousarray(
            ws.reshape(KC, 128, 8, VT).transpose(2, 1, 0, 3), dtype=NPBF)
        m["boutb"] = np.ascontiguousarray(
            bout[VS * c:VS * (c + 1)].reshape(8, VT), dtype=f)
        in_maps.append(m)
    return in_maps


def kernel(idx, tok_emb, pos_emb, Wq, Wk, Wv, W1, b1, W2, b2, Wout, bout):
    if "nc" not in _STATE:
        _STATE["nc"] = _build()
    nc = _STATE["nc"]

    in_maps = make_in_maps(idx, tok_emb, pos_emb, Wq, Wk, Wv, W1, b1, W2,
                           b2, Wout, bout)
    res = bass_utils.run_bass_kernel_spmd(nc, in_maps,
                                          core_ids=list(range(N_CORES)))
    _STATE["last_results"] = res

    out = np.empty((B * T, V), dtype=np.float32)
    for c in range(N_CORES):
        lg = res.results[c]["logits"]             # [8, 8, 512, 500] bf16
        out[:, VS * c:VS * (c + 1)] = (
            np.asarray(lg).transpose(0, 2, 1, 3).reshape(B * T, VS)
            .astype(np.float32))
    return out.reshape(B, T, V)


# revision 27
# speedup vs baseline: 4.7932x; 4.3134x over previous
"""Single-layer transformer LM head kernel for 8 Trainium2 NeuronCores.

Model (B=2, T=2048, D=1024, V=32000):
    x = tok_emb[idx] + pos_emb
    x = x + 0.125 * causal_attn(x@Wq, x@Wk, x@Wv)
    x = x + gelu(x@W1 + b1)@W2 + b2
    out = x@Wout + bout

Sharding (one uniform SPMD program on 8 cores):
  - trunk token-parallel: core c owns 512 tokens (batch c//4, block c%4)
  - K/V for the whole batch-sequence are recomputed locally on every core
    (cheaper than the 4-rank AllGather, measured): the host feeds each core
    the full-batch embeddings with the 512-token blocks ROTATED so the
    core's own block is always first -- this keeps every access pattern
    static/uniform across cores; causality lives in a per-core additive
    mask input built for the rotated order.
  - final-hidden AllGather across all 8 cores, split in two halves so the
    first half overlaps the tail of the MLP down-projection
  - logits vocab-parallel: each core does all 4096 tokens x 4000 vocab cols

All activations are kept in transposed [d, tokens] layout so every matmul
(lhsT.T @ rhs, contraction on partitions) is expressed without on-chip
transposes; attention scores are computed transposed [tk, tq] so the
softmax normalizer comes from a ones-vector matmul.

Matmul operands, the final-hidden AllGather, and all weight inputs are
bfloat16 (same PE rate as fp32r, half the HBM/DMA and SBUF traffic);
accumulation (PSUM), softmax statistics, residual masters and the logits
output stay fp32.  K and V stay resident in SBUF in exactly the layout
attention consumes (K-proj emits [d, tok], V-proj emits [tok, d]) -- no
DRAM round-trip.  Host folds pos_emb into the token embeddings, so the
kernel's embedding phase is a pure DMA.
"""
import numpy as np
import ml_dtypes
import concourse.bass as bass
import concourse.bacc as bacc
import concourse.tile as tile
from concourse import bass_utils, mybir
from contextlib import ExitStack

F32 = mybir.dt.float32
BF = mybir.dt.bfloat16
AF = mybir.ActivationFunctionType
OP = mybir.AluOpType
NPBF = ml_dtypes.bfloat16

N_CORES = 8
B, T, D, DH, V = 2, 2048, 1024, 4096, 32000
TB = T // 4            # 512 tokens per core
VS = V // N_CORES      # 4000 vocab cols per core
VT = VS // 8           # 500 per n-tile
NV = V // VT           # 64 vocab tiles (full vocab, token-parallel logits)
KC = D // 128          # 8 contraction chunks of d_model
HC = DH // 128         # 32 chunks of d_hidden
NTK = T // 128         # 16 key chunks (whole batch-sequence)
SCALE = 1.0 / 32.0     # 1/sqrt(D)
MASK_NEG = -1.0e4
XH_ELEMS = (KC // 2) * 128 * TB   # half of one core's final-hidden block

KV_GROUPS = [[0, 1, 2, 3], [4, 5, 6, 7]]

_STATE = {}
_NO_COLL = False   # timing/sim variant: skip collectives


def _alloc_logits_pools(tc, st, lg):
    """Reserve the logits staging pools on the right SBUF side (so trunk
    pools on the left can pop in stack order underneath them)."""
    lp = st.enter_context(tc.tile_pool(name="lgp", bufs=1, side="right"))
    lg["wop"] = st.enter_context(tc.tile_pool(name="wop", bufs=3,
                                              side="right"))
    lg["bout_s"] = lp.tile([128, 8, VT], F32, name="bout_s")
    lg["wot0"] = lg["wop"].tile([128, KC, VT], BF, name="wot")


def _load_logits0(nc, io, lg):
    """Queue the n=0 logits weight loads; called at MLP start so the loads
    land well before the trunk finishes."""
    for n in range(8):
        nc.scalar.dma_start(
            lg["bout_s"][:, n, :],
            io["boutb"].ap()[n:n + 1, :].partition_broadcast(128))
    nc.sync.dma_start(lg["wot0"][:], io["woutb"].ap()[0])


def _trunk(nc, tc, io, dp, bounce_k, bounce_v, ag_k, ag_v,
           bounce_x1, bounce_x2, ag_x1, ag_x2, st, lg):
    """Token-parallel trunk; ends with the split final-hidden AllGather.

    Pool lifetimes are managed manually: kv/res0 (K,V,Q + x0 master) are
    released right after attention so the MLP fits; res1 (x1 masters)
    spans attention through the MLP."""
    kvp = tc.alloc_tile_pool(name="kv", bufs=1)
    r0p = tc.alloc_tile_pool(name="res0", bufs=1)
    if True:
        kT_s = kvp.tile([128, KC, T], BF)      # K in [d, tok] layout
        v_s = kvp.tile([128, NTK, D], BF)      # V in [tok, d] layout
        qT = kvp.tile([128, KC, TB], BF)
        x0f32 = r0p.tile([128, KC, TB], F32)
        rs_b = r0p.tile([128, TB], F32)

        # ---- own-block embeddings + K/V/Q projections.  K/V for the other
        # three blocks arrive via two 4-rank AllGathers that fly behind the
        # V/Q projections and the early scores iterations ----
        with tc.tile_pool(name="xfull", bufs=1) as xp, \
             tc.tile_pool(name="kvown", bufs=1) as kop:
            x0F = xp.tile([128, KC, TB], BF)
            kt_own = kop.tile([128, KC, TB], BF)
            v_own = kop.tile([128, 4, D], BF)
            with tc.tile_pool(name="wqp", bufs=1) as wqp:
                wq_s = wqp.tile([128, KC, KC, 128], BF)
                with tc.tile_pool(name="wvp", bufs=1) as wvp, \
                     tc.tile_pool(name="wkp", bufs=1) as wkp, \
                     tc.tile_pool(name="ps_kv", bufs=6, space="PSUM") as pskv:
                    wk_s = wkp.tile([128, KC, KC, 128], BF)
                    wvr_s = wvp.tile([128, KC, D], BF)
                    # first matmul needs xt + wk: queue those first,
                    # wvr/wq on the other HWDGE queue
                    nc.sync.dma_start(
                        x0F[:], io["xt"].ap().rearrange("k p t -> p k t"))
                    for m in range(KC):
                        nc.scalar.dma_start(wk_s[:, m], io["wk"].ap()[:, m])
                    nc.sync.dma_start(wvr_s[:], io["wvr"].ap())
                    nc.scalar.dma_start(wq_s[:], io["wq"].ap())

                    # K projection (own block) -> bounce -> AllGather
                    for m in range(KC):
                        ps = pskv.tile([128, TB], F32, name="ps_kv")
                        for k in range(KC):
                            nc.tensor.matmul(
                                ps[:], wk_s[:, m, k, :], x0F[:, k, :],
                                start=(k == 0), stop=(k == KC - 1))
                        nc.vector.tensor_copy(kt_own[:, m, :], ps[:])
                    nc.gpsimd.dma_start(
                        bounce_k[:].rearrange("(k p t) -> p k t",
                                              k=KC, p=128),
                        kt_own[:])
                    if not _NO_COLL:
                        nc.gpsimd.collective_compute(
                            "AllGather", OP.bypass, replica_groups=KV_GROUPS,
                            ins=[bounce_k.opt()], outs=[ag_k.opt()])

                    # V projection (own block) -> bounce -> AllGather
                    for tc_ in range(4):
                        for h in range(2):
                            ps = pskv.tile([128, 512], F32, name="ps_kv")
                            for k in range(KC):
                                nc.tensor.matmul(
                                    ps[:], x0F[:, k, bass.ts(tc_, 128)],
                                    wvr_s[:, k, bass.ts(h, 512)],
                                    start=(k == 0), stop=(k == KC - 1))
                            nc.vector.tensor_copy(
                                v_own[:, tc_, bass.ts(h, 512)], ps[:])
                    nc.gpsimd.dma_start(
                        bounce_v[:].rearrange("(c p d) -> p c d", c=4, p=128),
                        v_own[:])
                    if not _NO_COLL:
                        nc.gpsimd.collective_compute(
                            "AllGather", OP.bypass, replica_groups=KV_GROUPS,
                            ins=[bounce_v.opt()], outs=[ag_v.opt()])

                    # gathered K/V -> SBUF (group member i = global block
                    # i); issued now so the loads run during Q projection
                    for i in range(4):
                        nc.gpsimd.dma_start(
                            kT_s[:, :, bass.ts(i, TB)],
                            ag_k[i].rearrange("(k p t) -> p k t",
                                              k=KC, p=128))
                    for i in range(4):
                        nc.gpsimd.dma_start(
                            v_s[:, 4 * i:4 * i + 4, :],
                            ag_v[i].rearrange("(c p d) -> p c d",
                                              c=4, p=128))

                    # Q projection (own block) while the AllGathers fly
                    for m in range(KC):
                        ps = pskv.tile([128, TB], F32, name="ps_kv")
                        for k in range(KC):
                            nc.tensor.matmul(
                                ps[:], wq_s[:, m, k, :], x0F[:, k, :],
                                start=(k == 0), stop=(k == KC - 1))
                        nc.vector.tensor_copy(qT[:, m, :], ps[:])
                    nc.vector.tensor_copy(x0f32[:], x0F[:])
                    # pre-load the ACT engine's Exp table so the scores
                    # loop doesn't stall on LoadActFuncSet
                    warm = r0p.tile([128, 1], F32, name="warm")
                    nc.vector.memset(warm[:], 0.0)
                    nc.scalar.activation(warm[:], warm[:], AF.Exp,
                                         scale=1.0)

        # ---------- attention (scores transposed: sT[tk, tq]) ----------
        if lg is not None:
            _alloc_logits_pools(tc, st, lg)
        r1p = tc.alloc_tile_pool(name="res1", bufs=1, side="right")
        x1bf = r1p.tile([128, KC, TB], BF)
        x1f32 = r1p.tile([128, KC, TB], F32)
        with tc.tile_pool(name="attn", bufs=1) as ap_, \
             tc.tile_pool(name="mskp", bufs=6) as mskp, \
             tc.tile_pool(name="stmp", bufs=3) as stp, \
             tc.tile_pool(name="ps_sc", bufs=2, space="PSUM") as ps_sc, \
             tc.tile_pool(name="ps_l", bufs=1, space="PSUM") as ps_lp, \
             tc.tile_pool(name="ps_o", bufs=2, space="PSUM") as ps_op:
            attnT = ap_.tile([128, NTK, TB], BF)
            ones_f32 = ap_.tile([128, 1], F32)
            nc.vector.memset(ones_f32[:], 1.0)
            ones_s = ap_.tile([128, 1], BF)
            nc.vector.tensor_copy(ones_s[:], ones_f32[:])
            ps_l = ps_lp.tile([1, TB], F32)

            for tkc in range(NTK):
                msk = mskp.tile([128, TB], F32, name="msk")
                nc.sync.dma_start(msk[:], io["mask"].ap()[tkc])
                ps = ps_sc.tile([128, TB], F32, name="ps_s")
                for k in range(KC):
                    nc.tensor.matmul(ps[:], kT_s[:, k, bass.ts(tkc, 128)],
                                     qT[:, k, :],
                                     start=(k == 0), stop=(k == KC - 1))
                stmp = stp.tile([128, TB], F32, name="stmp")
                nc.vector.tensor_tensor(out=stmp[:], in0=ps[:], in1=msk[:],
                                        op=OP.add)
                nc.scalar.activation(attnT[:, tkc, :], stmp[:], AF.Exp,
                                     scale=SCALE)
                nc.tensor.matmul(ps_l[:], ones_s[:], attnT[:, tkc, :],
                                 start=(tkc == 0), stop=(tkc == NTK - 1))

            # rs = 0.125 / l, broadcast to all partitions via DRAM bounce
            rs_row = ap_.tile([1, TB], F32)
            nc.vector.reciprocal(rs_row[:], ps_l[:])
            rs_row2 = ap_.tile([1, TB], F32)
            nc.vector.tensor_scalar_mul(rs_row2[:], rs_row[:], 0.125)
            rs_dram = dp.tile([1, TB], F32, name="rs_dram")
            nc.sync.dma_start(rs_dram[:], rs_row2[:])
            nc.sync.dma_start(rs_b[:], rs_dram[:].partition_broadcast(128))

            # oT[dv, tq] = V.T @ attnT ; x1 = x0 + rs * oT
            for m in range(KC):
                ps = ps_op.tile([128, TB], F32, name="ps_av")
                for tkc in range(NTK):
                    nc.tensor.matmul(ps[:], v_s[:, tkc, bass.ts(m, 128)],
                                     attnT[:, tkc, :],
                                     start=(tkc == 0), stop=(tkc == NTK - 1))
                ot = stp.tile([128, TB], F32, name="otmp")
                nc.vector.tensor_tensor(out=ot[:], in0=ps[:], in1=rs_b[:],
                                        op=OP.mult)
                nc.vector.tensor_tensor(out=x1f32[:, m, :], in0=ot[:],
                                        in1=x0f32[:, m, :], op=OP.add)
                nc.vector.tensor_copy(x1bf[:, m, :], x1f32[:, m, :])

        r0p.release()
        kvp.release()

        # ---------- MLP ----------
        with tc.tile_pool(name="mlp", bufs=1, side="right") as mp, \
             tc.tile_pool(name="w1p", bufs=4, side="right") as w1p, \
             tc.tile_pool(name="w2p", bufs=2, side="right") as w2p, \
             tc.tile_pool(name="ps_h", bufs=6, space="PSUM") as ps_hp:
            if lg is not None:
                _load_logits0(nc, io, lg)
            b1_s = mp.tile([128, HC], F32)
            b2_s = mp.tile([128, KC], F32)
            nc.sync.dma_start(b1_s[:], io["b1t"].ap())
            nc.sync.dma_start(b2_s[:], io["b2t"].ap())
            hT = mp.tile([128, HC, TB], BF)
            for m in range(HC):
                w1t = w1p.tile([128, KC, 128], BF, name="w1t")
                nc.sync.dma_start(w1t[:], io["w1b"].ap()[m])
                ps = ps_hp.tile([128, TB], F32, name="ps_mlp")
                for k in range(KC):
                    nc.tensor.matmul(ps[:], w1t[:, k, :], x1bf[:, k, :],
                                     start=(k == 0), stop=(k == KC - 1))
                nc.scalar.activation(hT[:, m, :], ps[:], AF.Gelu,
                                     bias=b1_s[:, m:m + 1], scale=1.0)
            x2bf = mp.tile([128, KC, TB], BF)
            for m in range(KC):
                w2t = w2p.tile([128, HC, 128], BF, name="w2t")
                nc.scalar.dma_start(w2t[:], io["w2b"].ap()[m])
                ps = ps_hp.tile([128, TB], F32, name="ps_mlp")
                for k in range(HC):
                    nc.tensor.matmul(ps[:], w2t[:, k, :], hT[:, k, :],
                                     start=(k == 0), stop=(k == HC - 1))
                # x2 = (psum + b2) + x1
                nc.vector.scalar_tensor_tensor(
                    out=x2bf[:, m, :], in0=ps[:], scalar=b2_s[:, m:m + 1],
                    in1=x1f32[:, m, :], op0=OP.add, op1=OP.add)
                # split final-hidden AllGather: 1st half overlaps m=4..7
                if m == KC // 2 - 1:
                    nc.gpsimd.dma_start(
                        bounce_x1[:].rearrange("(k p t) -> p k t",
                                               k=KC // 2, p=128),
                        x2bf[:, :KC // 2, :])
                    if not _NO_COLL:
                        nc.gpsimd.collective_compute(
                            "AllGather", OP.bypass,
                            replica_groups=[list(range(N_CORES))],
                            ins=[bounce_x1.opt()], outs=[ag_x1.opt()])
            nc.gpsimd.dma_start(
                bounce_x2[:].rearrange("(k p t) -> p k t",
                                       k=KC // 2, p=128),
                x2bf[:, KC // 2:, :])
            if not _NO_COLL:
                nc.gpsimd.collective_compute(
                    "AllGather", OP.bypass,
                    replica_groups=[list(range(N_CORES))],
                    ins=[bounce_x2.opt()], outs=[ag_x2.opt()])
        r1p.release()


def _logits(nc, tc, io, dp, ag_x1, ag_x2, st, lg):
    """Vocab-parallel logits over the AllGathered final hidden states.
    Output is written bf16 (host upcasts)."""
    out_d = io["logits"]
    if lg is None:
        lg = {}
        _alloc_logits_pools(tc, st, lg)
        _load_logits0(nc, io, lg)
    wop, bout_s = lg["wop"], lg["bout_s"]
    with tc.tile_pool(name="xfp", bufs=1) as lp, \
         tc.tile_pool(name="outp", bufs=4) as outp, \
         tc.tile_pool(name="ps_lg", bufs=8, space="PSUM") as ps_lg:
        xf = lp.tile([128, N_CORES * KC, TB], BF)

        def load_xf(r):
            nc.sync.dma_start(
                xf[:, KC * r:KC * r + KC // 2, :],
                ag_x1[r].rearrange("(k p t) -> p k t", k=KC // 2, p=128))
            nc.sync.dma_start(
                xf[:, KC * r + KC // 2:KC * (r + 1), :],
                ag_x2[r].rearrange("(k p t) -> p k t", k=KC // 2, p=128))
        wot = lg["wot0"]
        for n in range(8):
            if n > 0:
                wot = wop.tile([128, KC, VT], BF, name="wot")
                nc.sync.dma_start(wot[:], io["woutb"].ap()[n])
            for r in range(N_CORES):
                if n == 0:
                    load_xf(r)
                for t4 in range(4):
                    ps = ps_lg.tile([128, VT], F32, name="ps_g")
                    for k in range(KC):
                        nc.tensor.matmul(
                            ps[:], xf[:, KC * r + k, bass.ts(t4, 128)],
                            wot[:, k, :],
                            start=(k == 0), stop=(k == KC - 1))
                    ot = outp.tile([128, VT], BF, name="og")
                    nc.vector.tensor_tensor(out=ot[:], in0=ps[:],
                                            in1=bout_s[:, n, :], op=OP.add)
                    nc.scalar.dma_start(
                        out_d.ap()[r, n, bass.ts(t4, 128), :], ot[:])


def _build(repeat=1, phases="full"):
    nc = bacc.Bacc("TRN2", target_bir_lowering=False, debug=False,
                   num_devices=N_CORES)

    # ---- kernel I/O (per-core shards prepared on host) ----
    io = {}
    def inp(name, shape, dt=BF):
        io[name] = nc.dram_tensor(name, shape, dt, kind="ExternalInput")
    inp("xt", [KC, 128, TB])
    inp("wq", [128, KC, KC, 128])
    inp("wk", [128, KC, KC, 128])
    inp("wvr", [128, KC, D])
    inp("w1b", [HC, 128, KC, 128])
    inp("b1t", [128, HC], F32)
    inp("w2b", [KC, 128, HC, 128])
    inp("b2t", [128, KC], F32)
    inp("woutb", [8, 128, KC, VT])
    inp("boutb", [8, VT], F32)
    inp("mask", [NTK, 128, TB], F32)
    io["logits"] = nc.dram_tensor("logits", [N_CORES, 8, TB, VT], BF,
                                  kind="ExternalOutput")

    with tile.TileContext(nc) as tc:
        with tc.tile_pool(name="dram", bufs=1, space="DRAM") as dp:
            for _ in range(repeat):  # repeat>1 is a timing-only variant
                bounce_k = dp.tile([KC * 128 * TB], BF, name="bounce_k")
                bounce_v = dp.tile([4 * 128 * D], BF, name="bounce_v")
                ag_k = dp.tile([4, KC * 128 * TB], BF, name="ag_k")
                ag_v = dp.tile([4, 4 * 128 * D], BF, name="ag_v")
                bounce_x1 = dp.tile([XH_ELEMS], BF, name="bounce_x1")
                bounce_x2 = dp.tile([XH_ELEMS], BF, name="bounce_x2")
                ag_x1 = dp.tile([N_CORES, XH_ELEMS], BF, name="ag_x1",
                                addr_space="Shared")
                ag_x2 = dp.tile([N_CORES, XH_ELEMS], BF, name="ag_x2",
                                addr_space="Shared")
                with ExitStack() as st:
                    lg = {} if phases == "full" else None
                    if phases in ("full", "trunk"):
                        _trunk(nc, tc, io, dp, bounce_k, bounce_v,
                               ag_k, ag_v, bounce_x1, bounce_x2,
                               ag_x1, ag_x2, st, lg)
                    if phases in ("full", "logits"):
                        _logits(nc, tc, io, dp, ag_x1, ag_x2, st, lg)

    nc.compile()
    return nc


def _prep_shared(Wq, Wk, Wv, W1, b1, W2, b2, pos_emb, Wout, bout):
    f = np.float32
    sh = {}
    sh["wq"] = np.ascontiguousarray(
        Wq.reshape(KC, 128, KC, 128).transpose(1, 2, 0, 3), dtype=NPBF)
    sh["wk"] = np.ascontiguousarray(
        Wk.reshape(KC, 128, KC, 128).transpose(1, 2, 0, 3), dtype=NPBF)
    sh["wvr"] = np.ascontiguousarray(
        Wv.reshape(KC, 128, D).transpose(1, 0, 2), dtype=NPBF)
    sh["w1b"] = np.ascontiguousarray(
        W1.reshape(KC, 128, HC, 128).transpose(2, 1, 0, 3), dtype=NPBF)
    sh["b1t"] = np.ascontiguousarray(b1.reshape(HC, 128).T, dtype=f)
    sh["w2b"] = np.ascontiguousarray(
        W2.reshape(HC, 128, KC, 128).transpose(2, 1, 0, 3), dtype=NPBF)
    sh["b2t"] = np.ascontiguousarray(b2.reshape(KC, 128).T, dtype=f)


    # global-order causal masks: core with block j has queries
    # TB*j + cc; key chunk tkc covers global keys 128*tkc + rr.
    pos = np.asarray(pos_emb[:T], dtype=f)
    masks = []
    rr = np.arange(128)[:, None]
    cc = np.arange(TB)[None, :]
    for j in range(4):
        m = np.empty((NTK, 128, TB), dtype=f)
        for tkc in range(NTK):
            gtk = 128 * tkc + rr
            m[tkc] = np.where(gtk <= TB * j + cc, 0.0, MASK_NEG)
        masks.append(m)
    return sh, pos, masks


def make_in_maps(idx, tok_emb, pos_emb, Wq, Wk, Wv, W1, b1, W2, b2,
                 Wout, bout):
    f = np.float32
    tok_emb = np.asarray(tok_emb, dtype=f)
    idx = np.asarray(idx)
    sh, pos, masks = _prep_shared(
        np.asarray(Wq, f), np.asarray(Wk, f), np.asarray(Wv, f),
        np.asarray(W1, f), np.asarray(b1, f), np.asarray(W2, f),
        np.asarray(b2, f), np.asarray(pos_emb, f), None, None)
    Wout = np.asarray(Wout, f)
    bout = np.asarray(bout, f)

    tok_full = [tok_emb[np.asarray(idx[b], dtype=np.int64)] for b in range(B)]
    in_maps = []
    for c in range(N_CORES):
        b, j = c // 4, c % 4
        own = tok_full[b][TB * j:TB * (j + 1)] + pos[TB * j:TB * (j + 1)]
        m = dict(sh)
        m["xt"] = np.ascontiguousarray(own.T.reshape(KC, 128, TB), dtype=NPBF)
        m["mask"] = masks[j]
        ws = Wout[:, VS * c:VS * (c + 1)]
        m["woutb"] = np.ascontiguousarray(
            ws.reshape(KC, 128, 8, VT).transpose(2, 1, 0, 3), dtype=NPBF)
        m["boutb"] = np.ascontiguousarray(
            bout[VS * c:VS * (c + 1)].reshape(8, VT), dtype=f)
        in_maps.append(m)
    return in_maps


def kernel(idx, tok_emb, pos_emb, Wq, Wk, Wv, W1, b1, W2, b2, Wout, bout):
    if "nc" not in _STATE:
        _STATE["nc"] = _build()
    nc = _STATE["nc"]

    in_maps = make_in_maps(idx, tok_emb, pos_emb, Wq, Wk, Wv, W1, b1, W2,
                           b2, Wout, bout)
    res = bass_utils.run_bass_kernel_spmd(nc, in_maps,
                                          core_ids=list(range(N_CORES)))
    _STATE["last_results"] = res

    out = np.empty((B * T, V), dtype=np.float32)
    for c in range(N_CORES):
        lg = res.results[c]["logits"]             # [8, 8, 512, 500] bf16
        out[:, VS * c:VS * (c + 1)] = (
            np.asarray(lg).transpose(0, 2, 1, 3).reshape(B * T, VS)
            .astype(np.float32))
    return out.reshape(B, T, V)
